# revision 2
# baseline (speedup 1.0000x reference)
"""FaceConvDemodulated — v5: compressed tables, per-TILE gathers on 4 queues.

out[n, o] = sum_{k,i} padded[nbr[n,k], i] * w_demod[o, i, k] + bias[o]

Key change vs baseline: instead of the A/B table split (2 gathers per
index to cover 50001 rows with int16 indices), the host builds a
COMPRESSED table per (core, super-batch): the <=4608 unique padded rows
that batch references, remapped to local indices 64..64+U-1 (rows 0..63
are zeros for dead-face index spreading). One row-mode SWDGE gather per
batch resolves every (face, tap) row -> halves GPSIMD descriptor
generation (the baseline's critical path) and halves gather DMA traffic.
Downstream, no A+B merge exists: per 128-face tile per tap, 2 PE
transposes -> PSUM, one PSUM->SBUF copy (alternating ScalarE/VectorE to
balance engines), 2 fp16 matmuls accumulate into the face-major PSUM.
Gathers round-robin across all 4 SWDGE queues for ring capacity and
transfer overlap.
"""

import numpy as np

N_FACES = 50000
C = 256
K = 9
PAD_SIZE = N_FACES + 1
N_CORES = 8
SHARD = 6272            # 49 * 128 faces per core; 8 * 6272 = 50176
N_PAD_TOTAL = N_CORES * SHARD
ZPAD = 64               # zero rows per table slot for dead-index spreading
SUPER = 512
SUPERS = [SUPER] * (SHARD // SUPER) + ([SHARD % SUPER] if SHARD % SUPER else [])
SLOT_ROWS = ZPAD + SUPER * K          # 4672 rows per compressed table slot
N_SLOTS = len(SUPERS)                 # 13
IDXCOLS = SHARD * K // 16             # 3528

_compiled = None


def _build():
    import concourse.mybir as mybir
    import concourse.tile as tile
    from concourse import bacc
    from concourse.masks import make_identity

    f32, f16 = mybir.dt.float32, mybir.dt.float16
    i16 = mybir.dt.int16

    nc = bacc.Bacc("TRN2", target_bir_lowering=False, debug=False,
                   num_devices=N_CORES, num_swdge_queues=4)

    d_table = nc.dram_tensor("table", [N_SLOTS * SLOT_ROWS, C], f16,
                             kind="ExternalInput")
    d_wT = nc.dram_tensor("wT", [2 * K * 128, C], f32, kind="ExternalInput")
    d_bias = nc.dram_tensor("bias", [128, C], f32, kind="ExternalInput")
    d_idx = nc.dram_tensor("idx", [128, IDXCOLS], i16, kind="ExternalInput")
    d_out = nc.dram_tensor("out", [SHARD, C], f32, kind="ExternalOutput")

    NT = 2 * K  # 18 weight tiles of [128, C]

    with tile.TileContext(nc) as tc:
        with (
            tc.tile_pool(name="const", bufs=1) as cpool,
            tc.tile_pool(name="wstream", bufs=3) as wpool,
            tc.tile_pool(name="gather", bufs=8) as gpool,
            tc.tile_pool(name="gtp", bufs=6) as gtpool,
            tc.tile_pool(name="outp", bufs=4) as opool,
            tc.tile_pool(name="psum", bufs=3, space="PSUM") as pspool,
            tc.tile_pool(name="psumt", bufs=4, space="PSUM") as ptpool,
            tc.tile_pool(name="wpsum", bufs=1, space="PSUM") as wps,
        ):
            # ---- constants / inputs ----
            idx = cpool.tile([128, IDXCOLS], i16)
            bias_sb = cpool.tile([128, C], f32)
            nc.sync.dma_start(out=idx[:], in_=d_idx[:])
            nc.sync.dma_start(out=bias_sb[:], in_=d_bias[:])

            # ---- weight demodulation ----
            ones = cpool.tile([128, 128], f32)
            nc.vector.memset(ones[:], 1.0)
            sq_ps = wps.tile([128, C], f32, space="PSUM")
            w32s = []
            for t in range(NT):
                w32 = wpool.tile([128, C], f32, tag="w32", bufs=NT)
                nc.sync.dma_start(out=w32[:],
                                  in_=d_wT[t * 128:(t + 1) * 128, :])
                sq = wpool.tile([128, C], f32, tag="sq")
                nc.vector.tensor_mul(out=sq[:], in0=w32[:], in1=w32[:])
                # psum[m, o] = sum_c sq[c, o] for every m (row-broadcast sum)
                nc.tensor.matmul(out=sq_ps[:], lhsT=ones[:], rhs=sq[:],
                                 start=(t == 0), stop=(t == NT - 1))
                w32s.append(w32)
            denom = cpool.tile([128, C], f32)
            eps = cpool.tile([128, 1], f32)
            nc.vector.memset(eps[:], 1e-8)
            nc.scalar.activation(denom[:], sq_ps[:],
                                 mybir.ActivationFunctionType.Sqrt,
                                 bias=eps[:])
            dcoef = cpool.tile([128, C], f32)
            nc.vector.reciprocal(dcoef[:], denom[:])
            w16 = cpool.tile([128, NT, C], f16)
            for t in range(NT):
                nc.vector.tensor_mul(out=w16[:, t, :], in0=w32s[t][:],
                                     in1=dcoef[:])

            # identity for PE transposes
            ident = cpool.tile([128, 128], f16)
            make_identity(nc, ident[:])

            # ---- main loop: one gather per 128-face tile, queues 0-3 ----
            col0 = 0
            row0 = 0
            gq = 0
            for b, sf in enumerate(SUPERS):
                nF = sf // 128
                for fb in range(nF):
                    nI = 128 * K
                    ncol = nI // 16
                    buf = gpool.tile([128, K, C], f16, tag="buf")
                    nc.gpsimd.dma_gather(
                        out_ap=buf[:], in_ap=d_table[b * SLOT_ROWS:
                                                     (b + 1) * SLOT_ROWS, :],
                        idxs_ap=idx[:, col0:col0 + ncol],
                        num_idxs=nI, num_idxs_reg=nI, elem_size=C,
                        transpose=False, single_packet=False,
                        queue_num=gq % 4)
                    gq += 1
                    col0 += ncol
                    ps = pspool.tile([128, C], f32, space="PSUM")
                    for k in range(K):
                        src = buf[:, k, :]                  # [face, i]
                        pst = ptpool.tile([128, C], f16, space="PSUM")
                        nc.tensor.transpose(pst[:, 0:128], src[:, 0:128],
                                            ident[:])
                        nc.tensor.transpose(pst[:, 128:256],
                                            src[:, 128:256], ident[:])
                        gt = gtpool.tile([128, C], f16)
                        if k % 2 == 0:
                            nc.scalar.activation(
                                gt[:], pst[:],
                                mybir.ActivationFunctionType.Copy)
                        else:
                            nc.vector.tensor_scalar_add(gt[:], pst[:], 0.0)
                        nc.tensor.matmul(out=ps[:], lhsT=gt[:, 0:128],
                                         rhs=w16[:, 2 * k, :],
                                         start=(k == 0), stop=False)
                        nc.tensor.matmul(out=ps[:], lhsT=gt[:, 128:256],
                                         rhs=w16[:, 2 * k + 1, :],
                                         start=False, stop=(k == K - 1))
                    ot = opool.tile([128, C], f32)
                    nc.vector.tensor_add(out=ot[:], in0=ps[:], in1=bias_sb[:])
                    nc.sync.dma_start(
                        out=d_out[row0 + fb * 128: row0 + (fb + 1) * 128, :],
                        in_=ot[:])
                row0 += sf

    nc.compile()
    return nc


def _host_prep(x, weight, bias, face_neighborhood, face_is_pad):
    """Layout prep: per-batch compressed tables + local wrapped indices."""
    x = np.asarray(x, np.float32)
    w = np.asarray(weight, np.float32)          # [O, I, 1, K]
    b = np.asarray(bias, np.float32)
    nbr = np.asarray(face_neighborhood).astype(np.int32)   # [N, K]
    pad = np.asarray(face_is_pad).astype(bool)

    # padded feature table, mirroring reference._pad_features
    rank = np.clip(np.cumsum(~pad) - 1, 0, x.shape[0] - 1)
    padded = x.astype(np.float16)[rank]
    padded[pad] = 0

    # transposed weights: row (k*256 + i) -> o
    wT = np.ascontiguousarray(
        np.transpose(w[:, :, 0, :], (2, 1, 0)).reshape(2 * K * 128, C))
    bias_t = np.ascontiguousarray(np.broadcast_to(b[None, :], (128, C)))

    nbr_pad = np.full((N_PAD_TOTAL, K), PAD_SIZE - 1, np.int32)
    nbr_pad[:N_FACES] = nbr
    live_face = np.zeros(N_PAD_TOTAL, bool)
    live_face[:N_FACES] = True

    in_maps = []
    for core in range(N_CORES):
        shard = nbr_pad[core * SHARD:(core + 1) * SHARD]      # [SHARD, K]
        shard_live = live_face[core * SHARD:(core + 1) * SHARD]
        table = np.zeros((N_SLOTS * SLOT_ROWS, C), np.float16)
        cols = []
        dead_cols = []
        r0 = 0
        for bi, sf in enumerate(SUPERS):
            blk = shard[r0:r0 + sf]                           # [sf, K]
            blive = np.broadcast_to(shard_live[r0:r0 + sf, None],
                                    (sf, K))
            # per-tile k-major order: tile ft, tap k, face f
            lst = np.ascontiguousarray(
                blk.reshape(sf // 128, 128, K).transpose(0, 2, 1)
            ).reshape(-1)
            lstlive = np.ascontiguousarray(
                blive.reshape(sf // 128, 128, K).transpose(0, 2, 1)
            ).reshape(-1)
            local = np.zeros(lst.shape[0], np.int32)
            if lstlive.any():
                uniq, inv = np.unique(lst[lstlive], return_inverse=True)
                assert len(uniq) <= SUPER * K
                table[bi * SLOT_ROWS + ZPAD:
                      bi * SLOT_ROWS + ZPAD + len(uniq)] = padded[uniq]
                local[lstlive] = ZPAD + inv
            cols.append(local.reshape(-1, 16))                # [nI/16, 16]
            dead_cols.append((~lstlive).reshape(-1, 16))
            r0 += sf
        flat = np.concatenate(cols, axis=0)                   # [IDXCOLS, 16]
        deadf = np.concatenate(dead_cols, axis=0)
        wrapped = flat.T                                      # [16, IDXCOLS]
        deadw = deadf.T
        a16f = np.tile(wrapped, (8, 1))                       # replicate x8
        deadt = np.tile(deadw, (8, 1))
        spread = ((np.arange(a16f.shape[1])[None, :]
                   + 16 * np.arange(128)[:, None]) % ZPAD)
        a16 = np.where(deadt, spread, a16f).astype(np.int16)
        in_maps.append({"table": table, "wT": wT, "bias": bias_t,
                        "idx": a16})
    return in_maps


def make_in_maps(inputs):
    return _host_prep(inputs["x"], inputs["weight"], inputs["bias"],
                      inputs["face_neighborhood"], inputs["face_is_pad"])


def kernel(x, weight, bias, face_neighborhood, face_is_pad, pad_size):
    global _compiled
    from concourse import bass_utils

    if _compiled is None:
        _compiled = _build()
    nc = _compiled

    in_maps = _host_prep(x, weight, bias, face_neighborhood, face_is_pad)
    res = bass_utils.run_bass_kernel_spmd(nc, in_maps,
                                          core_ids=list(range(N_CORES)))
    globals()["_last_results"] = res
    out = np.concatenate([r["out"] for r in res.results], axis=0)[:N_FACES]
    return np.ascontiguousarray(out.astype(np.float32))


# revision 3
# speedup vs baseline: 1.0493x; 1.0493x over previous
"""FaceConvDemodulated — host-expanded neighborhood stream, dense device pipeline.

out[n, o] = sum_{k,i} padded[nbr[n,k], i] * w_demod[o, i, k] + bias[o]
  where w_demod = weight * rsqrt(sum_{i,k} weight^2 + 1e-8)  (per output ch.)

Design: every device-side indexed-fetch mechanism tried (SWDGE
dma_gather row/transpose modes with int16-range-remapped compressed
tables, 1-4 queues; GPSIMD ap_gather over an SBUF feature-on-partition
table) left the kernel gather-bound, 1.4-4.8x the PE roofline of this
contraction (CoreSim bodies 181-612 us vs 94 us of fp16 matmul work).
The bytes argument closes the case: a compressed unique-row table in HBM
(~1.67x dedup at this N) is the same ~29 MB/core an expanded stream
costs, so on-device index resolution only ADDS per-index ucode cost over
streaming the expanded data. The expansion is therefore host input prep
(pure layout: np fancy-index + transpose, rebuilt from the raw inputs on
every kernel() call), stored feature-on-partition so each DMA'd tile IS
the matmul lhsT with the contraction dim on partitions:

    tape[c, j, n] = x_padded[nbr[face(n), tap(n)], 128*j + c]

The device pipeline is dense and compute-bound (CoreSim 128 us/core vs
234 us for the previous A/B-split gather baseline): stream the tape
(sequential DMA, 28.9 MB/core/exec, overlapped), demodulate weights on
device (fp16 squares, fp32 PSUM sum via ones-matmul, Sqrt+reciprocal),
run 2 fp16 matmuls per 128-face tile per tap accumulating in PSUM, add
bias on VectorE, write out in natural [face, o] fp32 layout. No PE
transposes, no PSUM->SBUF copies, no GPSIMD. 8-way data-parallel over
faces; weights/bias replicated per core.
"""

import numpy as np

N_FACES = 50000
C = 256
K = 9
PAD_SIZE = N_FACES + 1
N_CORES = 8
SHARD = 6272            # 49 * 128 faces per core; 8 * 6272 = 50176
N_PAD_TOTAL = N_CORES * SHARD
SUPER = 512
SUPERS = [SUPER] * (SHARD // SUPER) + ([SHARD % SUPER] if SHARD % SUPER else [])
TAPECOLS = SHARD * K * 2          # 112896 fp16 per partition row

_compiled = None


def _build():
    import concourse.mybir as mybir
    import concourse.tile as tile
    from concourse import bacc

    f32, f16 = mybir.dt.float32, mybir.dt.float16

    nc = bacc.Bacc("TRN2", target_bir_lowering=False, debug=False,
                   num_devices=N_CORES)

    d_tape = nc.dram_tensor("tape", [128, TAPECOLS], f16,
                            kind="ExternalInput")
    d_wT = nc.dram_tensor("wT", [2 * K * 128, C], f16, kind="ExternalInput")
    d_bias = nc.dram_tensor("bias", [128, C], f32, kind="ExternalInput")
    d_out = nc.dram_tensor("out", [SHARD, C], f32, kind="ExternalOutput")

    NT = 2 * K

    with tile.TileContext(nc) as tc:
        with (
            tc.tile_pool(name="const", bufs=1) as cpool,
            tc.tile_pool(name="wstream", bufs=2) as wpool,
            tc.tile_pool(name="stream", bufs=3) as gpool,
            tc.tile_pool(name="outp", bufs=4) as opool,
            tc.tile_pool(name="psum", bufs=4, space="PSUM") as pspool,
            tc.tile_pool(name="wpsum", bufs=1, space="PSUM") as wps,
        ):
            bias_sb = cpool.tile([128, C], f32)
            nc.sync.dma_start(out=bias_sb[:], in_=d_bias[:])

            # ---- weight demodulation (fp16 weights, fp32 psum) ----
            w16r = cpool.tile([128, NT, C], f16)
            for t in range(NT):
                nc.sync.dma_start(out=w16r[:, t, :],
                                  in_=d_wT[t * 128:(t + 1) * 128, :])
            ones = cpool.tile([128, 128], f16)
            nc.vector.memset(ones[:], 1.0)
            sq_ps = wps.tile([128, C], f32, space="PSUM")
            for t in range(NT):
                sq = wpool.tile([128, C], f16, tag="sq")
                nc.vector.tensor_mul(out=sq[:], in0=w16r[:, t, :],
                                     in1=w16r[:, t, :])
                nc.tensor.matmul(out=sq_ps[:], lhsT=ones[:], rhs=sq[:],
                                 start=(t == 0), stop=(t == NT - 1))
            denom = cpool.tile([128, C], f32)
            eps = cpool.tile([128, 1], f32)
            nc.vector.memset(eps[:], 1e-8)
            nc.scalar.activation(denom[:], sq_ps[:],
                                 mybir.ActivationFunctionType.Sqrt,
                                 bias=eps[:])
            dcoef = cpool.tile([128, C], f32)
            nc.vector.reciprocal(dcoef[:], denom[:])
            dcoef16 = cpool.tile([128, C], f16)
            nc.scalar.activation(dcoef16[:], dcoef[:],
                                 mybir.ActivationFunctionType.Copy)
            w16 = cpool.tile([128, NT, C], f16)
            for t in range(NT):
                nc.vector.tensor_mul(out=w16[:, t, :], in0=w16r[:, t, :],
                                     in1=dcoef16[:])

            # ---- stream the tape, matmul per tile-tap ----
            col0 = 0
            row0 = 0
            for sf in SUPERS:
                nI = sf * K
                nF = sf // 128
                buf = gpool.tile([128, 2, nI], f16, tag="buf")
                nc.sync.dma_start(out=buf[:],
                                  in_=d_tape[:, col0:col0 + nI * 2])
                for fb in range(nF):
                    ps = pspool.tile([128, C], f32, space="PSUM")
                    for k in range(K):
                        n0 = k * sf + fb * 128
                        nc.tensor.matmul(out=ps[:],
                                         lhsT=buf[:, 0, n0:n0 + 128],
                                         rhs=w16[:, 2 * k, :],
                                         start=(k == 0), stop=False)
                        nc.tensor.matmul(out=ps[:],
                                         lhsT=buf[:, 1, n0:n0 + 128],
                                         rhs=w16[:, 2 * k + 1, :],
                                         start=False, stop=(k == K - 1))
                    ot = opool.tile([128, C], f32)
                    nc.vector.tensor_add(out=ot[:], in0=ps[:],
                                         in1=bias_sb[:])
                    nc.sync.dma_start(
                        out=d_out[row0 + fb * 128: row0 + (fb + 1) * 128, :],
                        in_=ot[:])
                col0 += nI * 2
                row0 += sf

    nc.compile()
    return nc


def _host_prep(x, weight, bias, face_neighborhood, face_is_pad):
    """Layout prep: padded table, per-core lhsT-layout neighborhood tape."""
    x = np.asarray(x, np.float32)
    w = np.asarray(weight, np.float32)
    b = np.asarray(bias, np.float32)
    nbr = np.asarray(face_neighborhood).astype(np.int32)
    pad = np.asarray(face_is_pad).astype(bool)

    rank = np.clip(np.cumsum(~pad) - 1, 0, x.shape[0] - 1)
    padded = x.astype(np.float16)[rank]
    padded[pad] = 0

    wT = np.ascontiguousarray(
        np.transpose(w[:, :, 0, :], (2, 1, 0)).reshape(2 * K * 128, C)
    ).astype(np.float16)
    bias_t = np.ascontiguousarray(np.broadcast_to(b[None, :], (128, C)))

    nbr_pad = np.full((N_PAD_TOTAL, K), PAD_SIZE - 1, np.int32)
    nbr_pad[:N_FACES] = nbr

    in_maps = []
    for core in range(N_CORES):
        shard = nbr_pad[core * SHARD:(core + 1) * SHARD]      # [SHARD, K]
        tape = np.empty((128, TAPECOLS), np.float16)
        col0 = 0
        r0 = 0
        for sf in SUPERS:
            blk = shard[r0:r0 + sf]                           # [sf, K]
            lst = blk.T.reshape(-1)                           # k-major [nI]
            rows = padded[lst]                                # [nI, 256]
            # tape[c, j, n] = rows[n, 128*j + c]
            t = rows.reshape(-1, 2, 128).transpose(2, 1, 0)   # [128, 2, nI]
            nI = sf * K
            tape[:, col0:col0 + nI * 2] = t.reshape(128, nI * 2)
            col0 += nI * 2
            r0 += sf
        in_maps.append({"tape": tape, "wT": wT, "bias": bias_t})
    return in_maps


def make_in_maps(inputs):
    return _host_prep(inputs["x"], inputs["weight"], inputs["bias"],
                      inputs["face_neighborhood"], inputs["face_is_pad"])


def kernel(x, weight, bias, face_neighborhood, face_is_pad, pad_size):
    global _compiled
    from concourse import bass_utils

    if _compiled is None:
        _compiled = _build()
    nc = _compiled

    in_maps = _host_prep(x, weight, bias, face_neighborhood, face_is_pad)
    res = bass_utils.run_bass_kernel_spmd(nc, in_maps,
                                          core_ids=list(range(N_CORES)))
    globals()["_last_results"] = res
    out = np.concatenate([r["out"] for r in res.results], axis=0)[:N_FACES]
    return np.ascontiguousarray(out.astype(np.float32))
